# revision 32
# baseline (speedup 1.0000x reference)
"""Multi-head attention with RoPE on 8 Trainium2 NeuronCores.

Problem: B=4, L=2048, D=1024, H=16 heads of dim 64, fp32, full (non-causal)
softmax attention with concatenated-halves RoPE on q and k.

Sharding: tensor-parallel over heads. Each of the 8 cores owns 2 heads:
 - computes q/k/v projections for its heads only (W_qkv column slice),
 - runs attention for its 2 heads x 4 batches,
 - computes a rank-128 partial of the output projection (W_proj row slice).
The host sums the 8 partial outputs (the only cross-core reduction).

On-core layout choices:
 - q, k are produced FEATURE-major ([head_dim, tokens]) directly by the QKV
   GEMM (weights pre-transposed/permuted on host), so the QK^T matmul needs
   no transposes. RoPE's even/odd feature split is pre-applied as a row
   permutation of W_q/W_k, so RoPE becomes 3 full-width DVE ops plus a
   32-partition-block swap done with SBUF->SBUF DMA.
 - scores are computed TRANSPOSED ([k_tokens, q_tokens]); softmax exp runs on
   ACT (scale folded into W_q on the host); the denominator comes free as an
   extra all-ones column appended to v in the p@v matmul.
 - v is produced feature-major then PE-transposed to token-major.
 - matmul operands are fp16 (PE streams 1 cycle/row vs 2 for fp32/fp32r);
   all accumulation stays fp32 in PSUM, softmax/rope/normalization math is
   fp32. Partial outputs ship fp16 (halves store DMA); host sums in fp64.
 - emission is software-pipelined at sub-chunk granularity: phase1 (QKV+rope)
   of batch b+1 and the output projection of batch b are queued as small
   "filler" units and injected between attention kj-steps, so the PE never
   starves while ACT exp drains.
"""

import sys

for _p in ("/opt/trn_rl_repo",):
    if _p not in sys.path:
        sys.path.insert(0, _p)

from collections import deque

import numpy as np
import concourse.bass as bass
import concourse.mybir as mybir
from concourse import bacc
from concourse.tile import TileContext
from concourse.bass_utils import run_bass_kernel_spmd
from concourse.masks import make_identity

F32 = mybir.dt.float32
F16 = mybir.dt.float16

B, L, D = 4, 2048, 1024
H, HD = 16, 64
NCORES = 8
HPC = H // NCORES  # 2 heads per core
TOK = B * L
BLK = 512  # gemm moving-dim block
QBLK = 512  # attention query block (one PSUM bank of fp32 output)
NBLK = L // BLK  # 4
NQB = L // QBLK  # 4
KT = D // 128  # 8 contraction tiles for the qkv projection
NKJ = L // 128  # 16 key tiles per batch
ROPE_BASE = 10000.0

Exp = mybir.ActivationFunctionType.Exp


class _Ctx:
    pass


def _build_program():
    nc = bacc.Bacc("TRN2", target_bir_lowering=False, debug=False)

    c = _Ctx()
    c.nc = nc
    c.xt_d = nc.dram_tensor("xt", [D, TOK], F16, kind="ExternalInput")
    c.wqk_d = nc.dram_tensor("wqk", [D, 256], F16, kind="ExternalInput")
    c.wv_d = nc.dram_tensor("wv", [D, 128], F16, kind="ExternalInput")
    c.wp_d = nc.dram_tensor("wp", [128, D], F16, kind="ExternalInput")
    c.cc_d = nc.dram_tensor("cc", [128, L], F32, kind="ExternalInput")
    c.ssw_d = nc.dram_tensor("ssw", [128, L], F32, kind="ExternalInput")
    c.out_d = nc.dram_tensor("out", [B, D, L], F16, kind="ExternalOutput")

    with TileContext(nc) as tc:
        with (
            tc.tile_pool(name="singles", bufs=1) as singles,
            tc.tile_pool(name="xin", bufs=2) as xin,
            tc.tile_pool(name="batch", bufs=2) as batch,
            tc.tile_pool(name="rope", bufs=2) as rope,
            tc.tile_pool(name="pexp", bufs=4) as pexp,
            tc.tile_pool(name="norm", bufs=4) as norm,
            tc.tile_pool(name="outp", bufs=4) as outp,
            tc.tile_pool(name="ps_g", bufs=2, space="PSUM") as ps_g,
            tc.tile_pool(name="ps_s", bufs=2, space="PSUM") as ps_s,
            tc.tile_pool(name="ps_o", bufs=2, space="PSUM") as ps_o,
        ):
            c.xin, c.batch, c.rope = xin, batch, rope
            c.pexp, c.norm, c.outp = pexp, norm, outp
            c.ps_g, c.ps_s, c.ps_o = ps_g, ps_s, ps_o

            # resident weights / tables. Emission order = DMA priority:
            # the first qkv matmul needs only wqk + x[b0, kd0].
            c.wqk_sb = singles.tile([128, KT, 256], F16, tag="wqk")
            nc.sync.dma_start(
                out=c.wqk_sb[:], in_=c.wqk_d[:, :].rearrange("(k p) e -> p k e", p=128)
            )
            c.bt = {}
            _xload(c, 0)  # batch-0 x, split per contraction tile
            c.cc_sb = singles.tile([128, L], F32, tag="cc")
            nc.sync.dma_start(out=c.cc_sb[:], in_=c.cc_d[:, :])
            c.ssw_sb = singles.tile([128, L], F32, tag="ssw")
            nc.sync.dma_start(out=c.ssw_sb[:], in_=c.ssw_d[:, :])
            c.wv_sb = singles.tile([128, KT, 128], F16, tag="wv")
            nc.sync.dma_start(
                out=c.wv_sb[:], in_=c.wv_d[:, :].rearrange("(k p) e -> p k e", p=128)
            )
            c.wp_sb = singles.tile([128, D], F16, tag="wp")
            nc.sync.dma_start(out=c.wp_sb[:], in_=c.wp_d[:, :])
            c.ident = singles.tile([128, 128], F16, tag="ident")
            make_identity(nc, c.ident[:])

            # prologue: only batch-0 q/k gemms + rope run inline (they are
            # DMA-gated by the x load anyway); v gemms + v transposes are
            # deferred into chunk (0,0) as fillers so attention starts the
            # moment q/k are rotated.
            for blk in range(NBLK):
                for u in _phase1_units(c, 0, blk, parts="qk", swap_dve=True):
                    u()
            _xload(c, 1)  # batch-1 x streams in during batch-0 attention

            fill = deque()
            vt0 = list(_vtrans_units(c, 0))
            for blk in range(NBLK):
                fill.extend(_phase1_units(c, 0, blk, parts="v"))
                fill.append(vt0[blk])
            for b in range(B):
                if b + 1 < B:
                    for blk in range(NBLK):
                        fill.extend(_phase1_units(c, b + 1, blk))
                    fill.extend(_vtrans_units(c, b + 1))
                for qi in range(NQB):
                    if qi == NQB - 1 and b + 2 < B:
                        _xload(c, b + 2)
                    # proj work is delayed by a full batch (the ao
                    # double-buffer allows exactly that) so the last batch's
                    # attention — which has no qkv filler left — stays fed.
                    if b > 0:
                        fill.extend(_phase3_units(c, b - 1, qi))
                    if b == B - 1 and qi > 0:
                        fill.extend(_phase3_units(c, b, qi - 1))
                    # chunk (0,0) pops a filler every kj (it must retire the
                    # v/vtrans units fast enough to feed its own av matmuls)
                    # and runs a deeper av lag to cover the v-transpose chain;
                    # the last batch also drains every kj so little proj work
                    # trails the final exp.
                    first = (b, qi) == (0, 0)
                    _phase2_chunk(
                        c, b, qi, fill,
                        every=1 if (first or b == B - 1) else 2,
                        lag=4 if first else 3,
                    )
            while fill:
                fill.popleft()()
            for u in _phase3_units(c, B - 1, NQB - 1):
                u()

    nc.compile()
    return nc


def _tiles(c, b):
    if b not in c.bt:
        t = _Ctx()
        t.q_ro = c.batch.tile([128, L], F16, tag="qro")
        t.k_ro = c.batch.tile([128, L], F16, tag="kro")
        t.v_fm = c.batch.tile([128, L], F16, tag="vfm")
        t.v0 = c.batch.tile([128, NKJ, 65], F16, tag="v0")
        t.v1 = c.batch.tile([128, NKJ, 65], F16, tag="v1")
        t.ao = c.batch.tile([128, L], F16, tag="ao")
        t.x_t = None
        c.bt[b] = t
    return c.bt[b]


def _xload(c, b):
    # x for batch b, feature-major, split per (contraction tile, half-L) so
    # the first gemm block is fully fed after 1/16 of the transfer.
    nc = c.nc
    t = _tiles(c, b)
    t.x_t = c.xin.tile([128, KT, L], F16, tag="x")
    for h in range(2):
        hs = slice(h * (L // 2), (h + 1) * (L // 2))
        for kd in range(KT):
            nc.sync.dma_start(
                out=t.x_t[:, kd, hs],
                in_=c.xt_d[
                    kd * 128 : (kd + 1) * 128,
                    b * L + h * (L // 2) : b * L + (h + 1) * (L // 2),
                ],
            )


def _phase1_units(c, b, blk, parts="qkv", swap_dve=False):
    """Yield ~8-matmul emission units: q-gemm+rope, k-gemm+rope, v-gemm.

    swap_dve: do the rope 32-row block swap with DVE copies instead of
    SBUF->SBUF DMA. DMA is cheaper in steady state, but during the initial
    bulk loads the swap descriptors would queue behind megabytes of x/table
    traffic, stalling the whole prologue."""
    nc = c.nc
    t = _tiles(c, b)
    ts = slice(blk * BLK, (blk + 1) * BLK)

    def qk_unit(wcol, dst):
        ps = c.ps_g.tile([128, BLK], F32, tag="g")
        for kd in range(KT):
            nc.tensor.matmul(
                ps[:],
                c.wqk_sb[:, kd, wcol : wcol + 128],
                t.x_t[:, kd, ts],
                start=(kd == 0),
                stop=(kd == KT - 1),
            )
        tmp_c = c.rope.tile([128, BLK], F32, tag="tc")
        nc.vector.tensor_mul(tmp_c[:], ps[:], c.cc_sb[:, ts])
        tmp_s = c.rope.tile([128, BLK], F32, tag="tsn")
        nc.vector.tensor_mul(tmp_s[:], ps[:], c.ssw_sb[:, ts])
        tmp_w = c.rope.tile([128, BLK], F32, tag="tw")
        for a, bb in ((0, 32), (32, 0), (64, 96), (96, 64)):
            if swap_dve:
                nc.vector.tensor_copy(tmp_w[a : a + 32, :], tmp_s[bb : bb + 32, :])
            else:
                nc.sync.dma_start(
                    out=tmp_w[a : a + 32, :], in_=tmp_s[bb : bb + 32, :]
                )
        nc.vector.tensor_add(dst[:, ts], tmp_c[:], tmp_w[:])

    def v_unit():
        psv = c.ps_g.tile([128, BLK], F32, tag="g")
        for kd in range(KT):
            nc.tensor.matmul(
                psv[:],
                c.wv_sb[:, kd, :],
                t.x_t[:, kd, ts],
                start=(kd == 0),
                stop=(kd == KT - 1),
            )
        nc.vector.tensor_copy(t.v_fm[:, ts], psv[:])

    if "q" in parts:
        yield lambda: qk_unit(0, t.q_ro)
    if "k" in parts:
        yield lambda: qk_unit(128, t.k_ro)
    if "v" in parts:
        yield v_unit


def _vtrans_units(c, b):
    """PE-transpose v to token-major, 4 tiles per emission unit so the
    2-deep psum WAW (transpose waits on DVE copy) never backs up the PE
    queue for long."""
    nc = c.nc
    t = _tiles(c, b)

    def unit(t0):
        if t0 == 0:
            nc.vector.memset(t.v0[:, :, 64], 1.0)
            nc.vector.memset(t.v1[:, :, 64], 1.0)
        for tt in range(t0, t0 + 4):
            pst = c.ps_g.tile([128, 128], F16, tag="g")
            nc.tensor.transpose(
                pst[:], t.v_fm[:, tt * 128 : (tt + 1) * 128], c.ident[:]
            )
            nc.vector.tensor_copy(t.v0[:, tt, 0:64], pst[:, 0:64])
            nc.vector.tensor_copy(t.v1[:, tt, 0:64], pst[:, 64:128])

    for t0 in range(0, NKJ, 4):
        yield lambda t0=t0: unit(t0)


def _phase2_chunk(c, b, qi, fill=None, every=2, lag=2):
    nc = c.nc
    t = _tiles(c, b)
    qs = slice(qi * QBLK, (qi + 1) * QBLK)
    o0 = c.ps_o.tile([65, QBLK], F32, tag="o")
    o1 = c.ps_o.tile([65, QBLK], F32, tag="o")
    ps = [None] * NKJ

    def av(kj):
        nc.tensor.matmul(
            o0[:], t.v0[:, kj, :], ps[kj][:, 0:QBLK],
            start=(kj == 0), stop=(kj == NKJ - 1),
        )
        nc.tensor.matmul(
            o1[:], t.v1[:, kj, :], ps[kj][:, QBLK : 2 * QBLK],
            start=(kj == 0), stop=(kj == NKJ - 1),
        )

    # av lags scores by two kj so the PE queue head never blocks on the
    # exp semaphore while ready scores work sits behind it.
    for kj in range(NKJ):
        ks = slice(kj * 128, (kj + 1) * 128)
        s_ps = c.ps_s.tile([128, 2 * QBLK], F32, tag="s")
        nc.tensor.matmul(
            s_ps[:, 0:QBLK], t.k_ro[0:64, ks], t.q_ro[0:64, qs],
            start=True, stop=True,
        )
        nc.tensor.matmul(
            s_ps[:, QBLK : 2 * QBLK],
            t.k_ro[64:128, ks],
            t.q_ro[64:128, qs],
            start=True,
            stop=True,
            tile_position=(64, 0),
        )
        # av before exp: exp(kj) recycles the p buffer of kj-4, so its
        # reader av(kj-4) must already be emitted when lag == 4
        if kj >= lag:
            av(kj - lag)
        p = c.pexp.tile([128, 2 * QBLK], F16, tag="p")
        nc.scalar.activation(p[:], s_ps[:], Exp)
        ps[kj] = p
        # inject one filler unit (qkv of next batch / proj of prev chunk)
        # into the PE stream while ACT chews on exp
        if fill and kj % every == every - 1:
            fill.popleft()()
    for kj in range(NKJ - lag, NKJ):
        av(kj)
    # early copies release the o-psum banks; recip/broadcast/multiply run
    # off the PE critical path.
    rb_full = c.norm.tile([128, QBLK], F32, tag="rbf")
    for o_ps, base in ((o0, 0), (o1, 64)):
        nc.vector.tensor_copy(t.ao[base : base + 64, qs], o_ps[0:64, :])
        stg = c.norm.tile([1, QBLK], F32, tag="stg")
        nc.vector.tensor_copy(stg[:], o_ps[64:65, :])
        r = c.norm.tile([1, QBLK], F32, tag="r")
        nc.vector.reciprocal_approx_fast(r[:], stg[:])
        if base == 0:
            nc.gpsimd.partition_broadcast(rb_full[0:64, :], r[:])
        else:
            rb1 = c.norm.tile([64, QBLK], F32, tag="rb")
            nc.gpsimd.partition_broadcast(rb1[:], r[:])
            nc.vector.tensor_copy(rb_full[64:128, :], rb1[:])
    nc.vector.tensor_mul(t.ao[:, qs], t.ao[:, qs], rb_full[:])


def _phase3_units(c, b, blk):
    """Output projection for one 512-token block, 2 e-tiles per unit (the
    two psum bufs) so back-to-back units never wait on the DVE drain."""
    nc = c.nc
    t = _tiles(c, b)
    ts = slice(blk * BLK, (blk + 1) * BLK)

    def unit(e0):
        for e in range(e0, e0 + 2):
            psf = c.ps_g.tile([128, BLK], F32, tag="g")
            nc.tensor.matmul(
                psf[:],
                c.wp_sb[:, e * 128 : (e + 1) * 128],
                t.ao[:, ts],
                start=True,
                stop=True,
            )
            o_sb = c.outp.tile([128, BLK], F16, tag="os")
            nc.vector.tensor_copy(o_sb[:], psf[:])
            nc.sync.dma_start(
                out=c.out_d[b, e * 128 : (e + 1) * 128, ts], in_=o_sb[:]
            )

    for e0 in range(0, D // 128, 2):
        yield lambda e0=e0: unit(e0)


_PROGRAM = None


def _program():
    global _PROGRAM
    if _PROGRAM is None:
        _PROGRAM = _build_program()
    return _PROGRAM


def _rope_tables():
    f = np.arange(32, dtype=np.float64)
    inv = ROPE_BASE ** (-2.0 * f / HD)
    t = np.arange(L, dtype=np.float64)
    ang = np.outer(inv, t)  # [32, L]
    cosT = np.cos(ang)
    sinT = np.sin(ang)
    cc = np.tile(cosT, (4, 1)).astype(np.float32)  # [128, L]
    ssw = np.concatenate([sinT, -sinT, sinT, -sinT], axis=0).astype(np.float32)
    return cc, ssw


def _prep_in_maps(x, W_qkv, W_proj):
    xt = np.ascontiguousarray(x.reshape(TOK, D).T).astype(np.float16)
    cc, ssw = _rope_tables()
    scale = HD**-0.5

    evens = np.arange(0, HD, 2)
    odds = np.arange(1, HD, 2)
    in_maps = []
    for c in range(NCORES):
        h0, h1 = HPC * c, HPC * c + 1
        rows_pair = np.concatenate(
            [h0 * HD + evens, h0 * HD + odds, h1 * HD + evens, h1 * HD + odds]
        )
        wq = (W_qkv[rows_pair, :].astype(np.float64) * scale).T  # [D, 128]
        wk = W_qkv[D + rows_pair, :].T  # [D, 128]
        wqk = np.concatenate([wq, wk], axis=1).astype(np.float16)
        rows_v = np.concatenate(
            [2 * D + h0 * HD + np.arange(HD), 2 * D + h1 * HD + np.arange(HD)]
        )
        wv = np.ascontiguousarray(W_qkv[rows_v, :].T).astype(np.float16)  # [D, 128]
        d_rows = np.concatenate([h0 * HD + np.arange(HD), h1 * HD + np.arange(HD)])
        wp = np.ascontiguousarray(W_proj[:, d_rows].T).astype(np.float16)  # [128, D]
        in_maps.append(
            {"xt": xt, "wqk": wqk, "wv": wv, "wp": wp, "cc": cc, "ssw": ssw}
        )
    return in_maps


def run(x, W_qkv, W_proj, trace=False):
    nc = _program()
    in_maps = _prep_in_maps(np.asarray(x), np.asarray(W_qkv), np.asarray(W_proj))
    res = run_bass_kernel_spmd(
        nc, in_maps, core_ids=list(range(NCORES)), trace=trace
    )
    acc = res.results[0]["out"].astype(np.float64)
    for c in range(1, NCORES):
        acc += res.results[c]["out"]
    full = np.transpose(acc, (0, 2, 1)).astype(np.float32)  # [B, L, D]
    return full, res


def kernel(x, W_qkv, W_proj):
    out, _ = run(x, W_qkv, W_proj, trace=False)
    return out
